# revision 19
# baseline (speedup 1.0000x reference)
"""Trainium2 Bass kernel for nn_CachedShapingFunctions (embedding_lookup).

out[b,t,w] = interp of lookup_table[:, w] at uniform-bucket position of
inputs[b,t,w].  Data-parallel over batch across 8 NeuronCores; the LUT is
replicated as a host-prepared bf16 (value, delta) pair table per waveshaper.

Per-core pipeline (shard flattened to [65536, 64], 64 super-tiles of
[128 part = 2 time-chunks x 64 w, 512 t]):
  - DMA in natural tiles, PE-transpose to waveshaper-on-partition layout
  - DVE: 7-op index pipeline (clamp, round(ic-0.5) floor trick, fraction)
  - GPSIMD ap_gather of bf16 (value, delta) pairs (the dominant cost,
    ~372us/tile -- command-bound at ~45ns/idx)
  - strided-partition extraction of the 1/16-dense gather output
    (split ACT/DVE), interpolation, PE-transpose back, DMA out

Scheduling: ap_gather contends catastrophically with concurrent DVE work
(shared POOL/DVE SBUF port), so all DVE/ACT work is packed into a serial
window between gathers via buffer reuse (bufs=1 pools) and a dummy
dependency-chaining op.  DMA and PE overlap the gather freely.
"""
import sys
import numpy as np

sys.path.insert(0, '/opt/trn_rl_repo')

import bass_rust
import concourse.bass as bass
import concourse.mybir as mybir
import concourse.tile as tile_mod
from concourse.tile import TileContext
from concourse.vector_clock import ScopedClock
from concourse import library_config

MIN_VALUE, MAX_VALUE = -3.0, 3.0
NB = 4096          # buckets
W = 64             # waveshapers
N_CORES = 8

# ---------------------------------------------------------------- patches --
# This walrus build accepts at most ONE sync-wait per instruction.  The Tile
# tail drain and scheduler can attach more; spill the excess onto nops.

_MAXW = 1

def _spill_waits(nc):
    for f in nc.m.functions:
        for bb in f.blocks:
            out = []
            for inst in list(bb.instructions):
                si = inst.sync_info
                if si is not None and len(si.on_wait) > _MAXW:
                    waits = list(si.on_wait)
                    spill = waits[:-_MAXW]
                    for i in range(0, len(spill), _MAXW):
                        nop = mybir.InstNoOp(
                            name=f"wspill_{inst.name}_{i}", ins=[], outs=[])
                        nop.engine = inst.engine
                        nop.sync_info = bass_rust.SyncInfo(
                            on_wait=spill[i:i + _MAXW], on_update=[])
                        out.append(nop)
                    inst.sync_info = bass_rust.SyncInfo(
                        on_wait=waits[-_MAXW:], on_update=list(si.on_update))
                out.append(inst)
            bb.instructions = out


def _patched_drain_and_barrier(self, tick_clock, wait_clock):
    nc = self.nc
    drain_inst = nc.sync.drain()
    wait_clock.add_sem_waits(
        drain_inst.ins, ScopedClock({None: tick_clock.global_clock}))
    si = drain_inst.ins.sync_info
    if si is not None and len(si.on_wait) > _MAXW:
        waits = list(si.on_wait)
        drain_inst.ins.sync_info = bass_rust.SyncInfo(
            on_wait=waits[:_MAXW], on_update=list(si.on_update))
        rest = waits[_MAXW:]
        for i in range(0, len(rest), _MAXW):
            nop = nc.sync.nop(hint="drain_wait_spill", nofuse=True)
            nop.ins.sync_info = bass_rust.SyncInfo(
                on_wait=rest[i:i + _MAXW], on_update=[])
    nc.all_engine_barrier()
    assert self.sems is not None
    popped = nc._tile_sem_poison_stack.pop()
    assert popped is self._sem_poison
    nc.clear_and_free_semaphores(list(self.sems.allocated().values()))
    nc.all_engine_barrier()


tile_mod.TileContext._drain_and_barrier = _patched_drain_and_barrier

# ----------------------------------------------------------------- kernel --

S = 512            # t-columns per transposed super-tile (per chunk)
TROWS = 2 * S      # natural t rows covered per super-tile (2 chunks)

F32 = mybir.dt.float32
I16 = mybir.dt.int16
BF16 = mybir.dt.bfloat16


def build_kernel(n_rows):
    """n_rows: flattened time rows per core (65536 full scale)."""
    assert n_rows % TROWS == 0
    n_tiles = n_rows // TROWS
    nc = bass.Bass()
    x_d = nc.dram_tensor("x", [n_rows, W], F32, kind="ExternalInput")
    tbl_d = nc.dram_tensor("tbl", [128, NB * 2], F32, kind="ExternalInput")
    id_d = nc.dram_tensor("ident", [128, 128], F32, kind="ExternalInput")
    mk_d = nc.dram_tensor("masks", [128, 32], F32, kind="ExternalInput")
    y_d = nc.dram_tensor("y", [n_rows, W], F32, kind="ExternalOutput")

    A = mybir.AluOpType

    with TileContext(nc) as tc:
        with (
            tc.tile_pool(name="const", bufs=1) as cpool,
            tc.tile_pool(name="io", bufs=3) as iop,
            tc.tile_pool(name="xt", bufs=2) as xtp,
            tc.tile_pool(name="sc", bufs=2) as scp,
            tc.tile_pool(name="sp", bufs=1) as spp,
            tc.tile_pool(name="on", bufs=2) as onp,
            tc.tile_pool(name="psi", bufs=4, space="PSUM") as psip,
            tc.tile_pool(name="pso", bufs=2, space="PSUM") as psop,
        ):
            tbl = cpool.tile([128, NB * 2], F32)
            ident = cpool.tile([128, 128], F32)
            masks = cpool.tile([128, 32], F32)
            nc.sync.dma_start(tbl[:, :], tbl_d[:, :])
            nc.sync.dma_start(ident[:, :], id_d[:, :])
            nc.sync.dma_start(masks[:, :], mk_d[:, :])
            nc.gpsimd.load_library(library_config.ap_gather)
            tbl3 = tbl[:, :].rearrange("p (n d) -> p n d", d=2)

            xnats = {}

            def emit_dma_in(i):
                xnat = iop.tile([128, 8 * W], F32, tag="xnat")
                in_ap = bass.AP(x_d, i * TROWS * W, [[W, 128], [128 * W, 8], [1, W]])
                nc.sync.dma_start(
                    xnat[:, :].rearrange("p (s w) -> p s w", s=8), in_ap)
                xnats[i] = xnat

            def emit_intrans_pe(i):
                """PE transposes for tile i (runs free, overlaps gather)."""
                xnat = xnats.pop(i)
                psts = []
                for k in range(4):
                    pst = psip.tile([128, 128], F32, tag="psin")
                    nc.tensor.transpose(
                        pst[:, :], xnat[:, 128 * k: 128 * k + 128], ident)
                    psts.append(pst)
                return psts

            def emit_intrans_act(i, psts):
                """PSUM -> xT copies (ACT, in the serial window)."""
                xT = xtp.tile([128, S], F32, tag="xT")
                for k in range(4):
                    nc.scalar.copy(xT[:, 128 * k: 128 * k + 128], psts[k][:, :])
                return xT

            def emit_idxprep(i, xT, idx):
                """DVE index pipeline (7 ops): writes idx (i16) + fraction ff."""
                ic = scp.tile([128, S], F32, tag="ic")
                icc = scp.tile([128, S], F32, tag="icc")
                tmp = scp.tile([128, S], F32, tag="tmp")
                ilf = scp.tile([128, S], F32, tag="ilf")
                icm = scp.tile([128, S], F32, tag="icm")
                ff = scp.tile([128, S], F32, tag="ff")
                nc.vector.tensor_scalar(ic[:, :], xT[:, :], 3.0, 682.5, A.add, A.mult)
                nc.vector.tensor_scalar(icc[:, :], ic[:, :], 0.0, 4095.0, A.max, A.min)
                # floor(icc) via round-to-nearest(icc - (0.5 - eps)); off-by-one
                # at segment boundaries is harmless (interpolation continuity).
                nc.vector.tensor_scalar(tmp[:, :], icc[:, :], 0.49999997, None, A.subtract)
                nc.vector.tensor_copy(idx[:, :], tmp[:, :])          # f32 -> i16 RNE
                nc.vector.tensor_copy(ilf[:, :], idx[:, :])          # i16 -> f32
                # upper-clamped ic for the fraction: keeps lower extrapolation
                # exact and forces out = T[4095] for ic >= 4095.
                nc.vector.tensor_scalar(icm[:, :], ic[:, :], 4095.0, None, A.min)
                nc.vector.tensor_tensor(ff[:, :], icm[:, :], ilf[:, :], A.subtract)
                return ff

            def emit_gather(i, idx):
                sparse = spp.tile([128, 16 * S * 2], F32, tag="sparse")
                sp3 = sparse[:, :].rearrange("p (n d) -> p n d", d=2)
                nc.gpsimd.ap_gather(sp3, tbl3, idx[:, :], channels=128,
                                    num_elems=NB, d=2, num_idxs=16 * S)
                return sparse

            def emit_extract_interp(i, sparse, ff, masks):
                """Compact the 1/16-dense gather output: partition p keeps
                pair slots m = 16n + (p%16).  16 masked multiply-accumulate
                merges in f32 (mask is 1.0/0.0), then interpolate."""
                pairs = scp.tile([128, S * 2], F32, tag="pairs")
                pr3 = pairs[:, :].rearrange("p (n q) -> p n q", q=2)
                sp32 = sparse[:, :].rearrange("p (n q) -> p n q", q=32)
                # masked zero-out (mask broadcast over n), then segmented
                # add-reduce over the 16 pair slots (15 zeros + the kept
                # pair -- exact in f32).
                mb = masks[:, :].unsqueeze(1).broadcast_to((128, S, 32))
                nc.vector.tensor_tensor(sp32, sp32, mb, A.mult)
                rin = sparse[:, :].rearrange(
                    "p (n q d) -> p n d q", q=16, d=2)
                nc.vector.tensor_reduce(
                    pr3, rin, mybir.AxisListType.X, A.add)
                outT = scp.tile([128, S], F32, tag="outT")
                nc.vector.tensor_tensor(outT[:, :], ff[:, :], pr3[:, :, 1], A.mult)
                nc.vector.tensor_tensor(outT[:, :], outT[:, :], pr3[:, :, 0], A.add)
                return outT

            def emit_out(i, outT):
                onat = onp.tile([128, 8 * W], F32, tag="onat")
                for k in range(4):
                    pst = psop.tile([128, 128], F32, tag="psout")
                    nc.tensor.transpose(
                        pst[:, :], outT[:, 128 * k: 128 * k + 128], ident)
                    nc.scalar.copy(onat[:, 128 * k: 128 * k + 128], pst[:, :])
                out_ap = bass.AP(y_d, i * TROWS * W, [[W, 128], [128 * W, 8], [1, W]])
                nc.sync.dma_start(
                    out_ap, onat[:, :].rearrange("p (s w) -> p s w", s=8))
                return onat

            # -------- warmup: prime two tiles of input + indices
            emit_dma_in(0)
            emit_dma_in(1)
            psts0 = emit_intrans_pe(0)
            xT0 = emit_intrans_act(0, psts0)
            idx0 = scp.tile([128, S], I16, tag="idx")
            ff0 = emit_idxprep(0, xT0, idx0)
            pend = (idx0, ff0)

            # -------- main loop: free-running pipeline (double buffers);
            # the gather paces everything, the rest overlaps it.
            for i in range(n_tiles):
                idx, ff = pend
                sparse = emit_gather(i, idx)
                if i + 2 < n_tiles:
                    emit_dma_in(i + 2)
                if i + 1 < n_tiles:
                    psts = emit_intrans_pe(i + 1)
                    xT = emit_intrans_act(i + 1, psts)
                    idxn = scp.tile([128, S], I16, tag="idx")
                    ffn = emit_idxprep(i + 1, xT, idxn)
                    pend = (idxn, ffn)
                outT = emit_extract_interp(i, sparse, ff, masks)
                emit_out(i, outT)

    from concourse.library_overlay import lower_extended_insts
    lower_extended_insts(nc)
    _spill_waits(nc)
    return nc


def make_table(lookup_table):
    lut = np.asarray(lookup_table, dtype=np.float32)          # [4096, 64]
    vu = np.concatenate([lut[1:], lut[-1:]], axis=0)          # T[min(i+1,4095)]
    delta = vu - lut                                          # f32 exact
    pair = np.stack([lut, delta], axis=-1)                    # [4096, 64, 2]
    tblw = np.ascontiguousarray(pair.transpose(1, 0, 2)).reshape(W, NB * 2)
    tbl128 = np.concatenate([tblw, tblw], axis=0)             # [128, 8192]
    return np.ascontiguousarray(tbl128)


def make_masks():
    p = np.arange(128)
    m = (p[:, None] % 16 == np.arange(16)[None, :]).astype(np.float32)
    return np.repeat(m, 2, axis=1)                            # [128, 32]


_CACHE = {}


def kernel(inputs, lookup_table):
    x = np.ascontiguousarray(np.asarray(inputs, dtype=np.float32))
    B, T, Wx = x.shape
    assert Wx == W
    per_core_b = B // N_CORES
    n_rows = per_core_b * T
    tbl = make_table(lookup_table)
    ident = np.eye(128, dtype=np.float32)
    masks = make_masks()

    if n_rows not in _CACHE:
        _CACHE[n_rows] = build_kernel(n_rows)
    nc = _CACHE[n_rows]

    from concourse import bass_utils
    shards = x.reshape(N_CORES, n_rows, W)
    in_maps = [{"x": shards[c], "tbl": tbl, "ident": ident, "masks": masks}
               for c in range(N_CORES)]
    res = bass_utils.run_bass_kernel_spmd(
        nc, in_maps, core_ids=list(range(N_CORES)))
    out = np.stack([res.results[c]["y"] for c in range(N_CORES)], axis=0)
    return out.reshape(B, T, W)
